# revision 10
# baseline (speedup 1.0000x reference)
"""v4: fp16 pipeline, DVE-only compute, pair-merged scans via a-zeroing.

Same sharding/layout as v3. New tricks vs v3:
- dram tensors are declared [PB, NBLK*T] (same memory) so 2-block spans are
  2D contiguous slices that tensor_tensor_scan accepts directly.
- Blocks 2..15 are processed as RB=2 pairs with ONE mul + ONE scan of 4096
  elements per pair. Correctness across the in-pair block boundary comes from
  zeroing a at the second block's first element: h[start] = g[start] + 0*state
  resets the recurrence exactly (a is only consumed by the scan).
- Block 0 ramps up in geometric chunks (256,256,512,1024) so the first scan
  starts as early as possible; block 1 is a single block; the last pair is
  scanned/stored in chained 1024-chunks to shorten the drain.
"""

import numpy as np

T, B, D = 2048, 16, 1024
NCORES = 8
DS = D // NCORES
NBLK = B
RB = 2
PB = 128

_cached = {}


def _build():
    import concourse.bacc as bacc
    import concourse.mybir as mybir
    import concourse.tile as tile

    f16 = mybir.dt.float16
    M, A = mybir.AluOpType.mult, mybir.AluOpType.add
    Copy = mybir.ActivationFunctionType.Copy
    nc = bacc.Bacc("TRN2", target_bir_lowering=False, debug=False, num_devices=NCORES)
    N = NBLK * T
    f_in = nc.dram_tensor("f_in", [PB, N], f16, kind="ExternalInput").ap()
    x_in = nc.dram_tensor("x_in", [PB, N], f16, kind="ExternalInput").ap()
    h_out = nc.dram_tensor("h_out", [PB, N], f16, kind="ExternalOutput").ap()

    with tile.TileContext(nc) as tc:
        with (
            tc.tile_pool(name="io", bufs=3) as io_pool,
            tc.tile_pool(name="b01", bufs=1) as b01_pool,
            tc.tile_pool(name="ap", bufs=2) as a_pool,
            tc.tile_pool(name="gp", bufs=3) as g_pool,
            tc.tile_pool(name="hp", bufs=3) as h_pool,
            tc.tile_pool(name="hd", bufs=1) as hd_pool,
        ):
            deferred = {}

            # --- block 0: geometric ramp-up ------------------------------
            h0 = hd_pool.tile([PB, T], f16, tag="hd0", name="hd0")
            pos = 0
            for i, L in enumerate((256, 256, 512, 1024)):
                sl = slice(pos, pos + L)
                fq = b01_pool.tile([PB, L], f16, tag=f"f0q{i}")
                nc.sync.dma_start(out=fq[:], in_=f_in[:, sl])
                xq = b01_pool.tile([PB, L], f16, tag=f"x0q{i}")
                nc.sync.dma_start(out=xq[:], in_=x_in[:, sl])
                aq = b01_pool.tile([PB, L], f16, tag=f"a0q{i}")
                nc.scalar.activation(aq[:], fq[:], Copy, bias=1.0, scale=-1.0)
                gq = b01_pool.tile([PB, L], f16, tag=f"g0q{i}")
                nc.vector.tensor_mul(gq[:], fq[:], xq[:])
                init = 0.0 if pos == 0 else h0[:, pos - 1 : pos]
                nc.vector.tensor_tensor_scan(h0[:, sl], aq[:], gq[:], init, M, A)
                pos += L
            deferred[0] = h0

            # --- block 1: single block -----------------------------------
            sl1 = slice(T, 2 * T)
            f1 = b01_pool.tile([PB, T], f16, tag="f1")
            nc.sync.dma_start(out=f1[:], in_=f_in[:, sl1])
            x1 = b01_pool.tile([PB, T], f16, tag="x1")
            nc.sync.dma_start(out=x1[:], in_=x_in[:, sl1])
            a1 = a_pool.tile([PB, T], f16, tag="a")
            nc.scalar.activation(a1[:], f1[:], Copy, bias=1.0, scale=-1.0)
            g1 = g_pool.tile([PB, T], f16, tag="g1")
            nc.vector.tensor_mul(g1[:], f1[:], x1[:])
            h1 = hd_pool.tile([PB, T], f16, tag="hd1", name="hd1")
            nc.vector.tensor_tensor_scan(h1[:], a1[:], g1[:], 0.0, M, A)
            deferred[1] = h1

            # --- pairs (2,3) .. (14,15) ----------------------------------
            W = RB * T  # 4096
            npairs = NBLK // RB
            for r in range(1, npairs):
                sl = slice(W * r, W * (r + 1))
                f_t = io_pool.tile([PB, W], f16, tag="f")
                nc.sync.dma_start(out=f_t[:], in_=f_in[:, sl])
                x_t = io_pool.tile([PB, W], f16, tag="x")
                nc.sync.dma_start(out=x_t[:], in_=x_in[:, sl])
                if r == npairs - 1:
                    # Sync ring idles after the final load: flush deferred
                    # block-0/1 stores there to fill the end DMA gap
                    for dblk, dh in deferred.items():
                        nc.sync.dma_start(
                            out=h_out[:, T * dblk : T * (dblk + 1)], in_=dh[:]
                        )
                a_t = a_pool.tile([PB, W], f16, tag="aw")
                nc.scalar.activation(a_t[:], f_t[:], Copy, bias=1.0, scale=-1.0)
                # zero a at the second block's first element: resets the
                # recurrence exactly at the in-pair block boundary
                nc.scalar.activation(
                    a_t[:, T : T + 1], f_t[:, T : T + 1], Copy, bias=0.0, scale=0.0
                )
                g_t = g_pool.tile([PB, W], f16, tag="gw")
                nc.vector.tensor_mul(g_t[:], f_t[:], x_t[:])
                h_t = h_pool.tile([PB, W], f16, tag="h")
                if r < npairs - 1:
                    nc.vector.tensor_tensor_scan(h_t[:], a_t[:], g_t[:], 0.0, M, A)
                    nc.scalar.dma_start(out=h_out[:, sl], in_=h_t[:])
                else:
                    # last pair: chained 1024-chunk scans + stores to
                    # shorten the pipeline drain
                    C = 1024
                    for q in range(W // C):
                        qsl = slice(C * q, C * (q + 1))
                        init = 0.0 if q == 0 else h_t[:, C * q - 1 : C * q]
                        nc.vector.tensor_tensor_scan(
                            h_t[:, qsl], a_t[:, qsl], g_t[:, qsl], init, M, A
                        )
                        nc.scalar.dma_start(
                            out=h_out[:, W * r + C * q : W * r + C * (q + 1)],
                            in_=h_t[:, qsl],
                        )
    nc.compile()
    return nc


def _get_nc():
    if "nc" not in _cached:
        _cached["nc"] = _build()
    return _cached["nc"]


def _shard(arr):
    """[T, B, D] -> per-core fp16 [DS, B*T] (partition-major), T reversed."""
    v = arr[::-1].transpose(2, 1, 0)  # [D, B, T] strided view, T reversed
    return [
        v[DS * c : DS * (c + 1)].astype(np.float16).reshape(DS, B * T)
        for c in range(NCORES)
    ]


def _run(f, x, trace=False):
    from concourse.bass_utils import run_bass_kernel_spmd

    f = np.asarray(f, dtype=np.float32)
    x = np.asarray(x, dtype=np.float32)
    assert f.shape == (T, B, D) and x.shape == (T, B, D)

    nc = _get_nc()
    f_shards = _shard(f)
    x_shards = _shard(x)
    in_maps = [{"f_in": f_shards[c], "x_in": x_shards[c]} for c in range(NCORES)]
    res = run_bass_kernel_spmd(nc, in_maps, core_ids=list(range(NCORES)), trace=trace)

    out = np.empty((T, B, D), dtype=np.float32)
    for c in range(NCORES):
        hc = res.results[c]["h_out"].reshape(DS, B, T)
        out[:, :, DS * c : DS * (c + 1)] = hc[:, :, ::-1].transpose(2, 1, 0)
    return out.reshape(T * B, D), res


def kernel(f, x):
    return _run(f, x, trace=False)[0]


# revision 12
# speedup vs baseline: 1.0163x; 1.0163x over previous
"""v4: fp16 pipeline, DVE-only compute, pair-merged scans via a-zeroing.

Same sharding/layout as v3. New tricks vs v3:
- dram tensors are declared [PB, NBLK*T] (same memory) so 2-block spans are
  2D contiguous slices that tensor_tensor_scan accepts directly.
- Blocks 2..15 are processed as RB=2 pairs with ONE mul + ONE scan of 4096
  elements per pair. Correctness across the in-pair block boundary comes from
  zeroing a at the second block's first element: h[start] = g[start] + 0*state
  resets the recurrence exactly (a is only consumed by the scan).
- Block 0 ramps up in geometric chunks (256,256,512,1024) so the first scan
  starts as early as possible; block 1 is a single block; the last pair is
  scanned/stored in chained 1024-chunks to shorten the drain.
"""

import numpy as np

T, B, D = 2048, 16, 1024
NCORES = 8
DS = D // NCORES
NBLK = B
RB = 2
PB = 128

_cached = {}


def _build():
    import concourse.bacc as bacc
    import concourse.mybir as mybir
    import concourse.tile as tile

    f16 = mybir.dt.float16
    M, A = mybir.AluOpType.mult, mybir.AluOpType.add
    Copy = mybir.ActivationFunctionType.Copy
    nc = bacc.Bacc("TRN2", target_bir_lowering=False, debug=False, num_devices=NCORES)
    N = NBLK * T
    f_in = nc.dram_tensor("f_in", [PB, N], f16, kind="ExternalInput").ap()
    x_in = nc.dram_tensor("x_in", [PB, N], f16, kind="ExternalInput").ap()
    h_out = nc.dram_tensor("h_out", [PB, N], f16, kind="ExternalOutput").ap()

    with tile.TileContext(nc) as tc:
        with (
            tc.tile_pool(name="io", bufs=3) as io_pool,
            tc.tile_pool(name="b01", bufs=1) as b01_pool,
            tc.tile_pool(name="ap", bufs=2) as a_pool,
            tc.tile_pool(name="gp", bufs=3) as g_pool,
            tc.tile_pool(name="hp", bufs=3) as h_pool,
            tc.tile_pool(name="hd", bufs=1) as hd_pool,
        ):
            deferred = {}

            # --- block 0: ramp-up (2 chunks: fewer DMA issues win over
            # finer granularity — each dma_start costs ~610ns of sequencer
            # issue time and ~3.5us to first data) ------------------------
            h0 = hd_pool.tile([PB, T], f16, tag="hd0", name="hd0")
            pos = 0
            for i, L in enumerate((512, 1536)):
                sl = slice(pos, pos + L)
                fq = b01_pool.tile([PB, L], f16, tag=f"f0q{i}")
                nc.sync.dma_start(out=fq[:], in_=f_in[:, sl])
                xq = b01_pool.tile([PB, L], f16, tag=f"x0q{i}")
                nc.sync.dma_start(out=xq[:], in_=x_in[:, sl])
                aq = b01_pool.tile([PB, L], f16, tag=f"a0q{i}")
                nc.scalar.activation(aq[:], fq[:], Copy, bias=1.0, scale=-1.0)
                gq = b01_pool.tile([PB, L], f16, tag=f"g0q{i}")
                nc.vector.tensor_mul(gq[:], fq[:], xq[:])
                init = 0.0 if pos == 0 else h0[:, pos - 1 : pos]
                nc.vector.tensor_tensor_scan(h0[:, sl], aq[:], gq[:], init, M, A)
                pos += L
            deferred[0] = h0

            # --- block 1: single block -----------------------------------
            sl1 = slice(T, 2 * T)
            f1 = b01_pool.tile([PB, T], f16, tag="f1")
            nc.sync.dma_start(out=f1[:], in_=f_in[:, sl1])
            x1 = b01_pool.tile([PB, T], f16, tag="x1")
            nc.sync.dma_start(out=x1[:], in_=x_in[:, sl1])
            a1 = a_pool.tile([PB, T], f16, tag="a")
            nc.scalar.activation(a1[:], f1[:], Copy, bias=1.0, scale=-1.0)
            g1 = g_pool.tile([PB, T], f16, tag="g1")
            nc.vector.tensor_mul(g1[:], f1[:], x1[:])
            h1 = hd_pool.tile([PB, T], f16, tag="hd1", name="hd1")
            nc.vector.tensor_tensor_scan(h1[:], a1[:], g1[:], 0.0, M, A)
            deferred[1] = h1

            # --- pairs (2,3) .. (14,15) ----------------------------------
            W = RB * T  # 4096
            npairs = NBLK // RB
            for r in range(1, npairs):
                sl = slice(W * r, W * (r + 1))
                f_t = io_pool.tile([PB, W], f16, tag="f")
                nc.sync.dma_start(out=f_t[:], in_=f_in[:, sl])
                x_t = io_pool.tile([PB, W], f16, tag="x")
                nc.sync.dma_start(out=x_t[:], in_=x_in[:, sl])
                if r == npairs - 1:
                    # Sync ring idles after the final load: flush deferred
                    # block-0/1 stores there to fill the end DMA gap
                    for dblk, dh in deferred.items():
                        nc.sync.dma_start(
                            out=h_out[:, T * dblk : T * (dblk + 1)], in_=dh[:]
                        )
                a_t = a_pool.tile([PB, W], f16, tag="aw")
                nc.scalar.activation(a_t[:], f_t[:], Copy, bias=1.0, scale=-1.0)
                # zero a at the second block's first element: resets the
                # recurrence exactly at the in-pair block boundary
                nc.scalar.activation(
                    a_t[:, T : T + 1], f_t[:, T : T + 1], Copy, bias=0.0, scale=0.0
                )
                g_t = g_pool.tile([PB, W], f16, tag="gw")
                nc.vector.tensor_mul(g_t[:], f_t[:], x_t[:])
                h_t = h_pool.tile([PB, W], f16, tag="h")
                if r < npairs - 1:
                    nc.vector.tensor_tensor_scan(h_t[:], a_t[:], g_t[:], 0.0, M, A)
                    nc.scalar.dma_start(out=h_out[:, sl], in_=h_t[:])
                else:
                    # last pair: chained chunk scans + stores, shrinking
                    # toward the end to shorten the pipeline drain
                    qpos = 0
                    for L in (1024, 1024, 1024, 512, 256, 256):
                        qsl = slice(qpos, qpos + L)
                        init = 0.0 if qpos == 0 else h_t[:, qpos - 1 : qpos]
                        nc.vector.tensor_tensor_scan(
                            h_t[:, qsl], a_t[:, qsl], g_t[:, qsl], init, M, A
                        )
                        nc.scalar.dma_start(
                            out=h_out[:, W * r + qpos : W * r + qpos + L],
                            in_=h_t[:, qsl],
                        )
                        qpos += L
    nc.compile()
    return nc


def _get_nc():
    if "nc" not in _cached:
        _cached["nc"] = _build()
    return _cached["nc"]


def _shard(arr):
    """[T, B, D] -> per-core fp16 [DS, B*T] (partition-major), T reversed."""
    v = arr[::-1].transpose(2, 1, 0)  # [D, B, T] strided view, T reversed
    return [
        v[DS * c : DS * (c + 1)].astype(np.float16).reshape(DS, B * T)
        for c in range(NCORES)
    ]


def _run(f, x, trace=False):
    from concourse.bass_utils import run_bass_kernel_spmd

    f = np.asarray(f, dtype=np.float32)
    x = np.asarray(x, dtype=np.float32)
    assert f.shape == (T, B, D) and x.shape == (T, B, D)

    nc = _get_nc()
    f_shards = _shard(f)
    x_shards = _shard(x)
    in_maps = [{"f_in": f_shards[c], "x_in": x_shards[c]} for c in range(NCORES)]
    res = run_bass_kernel_spmd(nc, in_maps, core_ids=list(range(NCORES)), trace=trace)

    out = np.empty((T, B, D), dtype=np.float32)
    for c in range(NCORES):
        hc = res.results[c]["h_out"].reshape(DS, B, T)
        out[:, :, DS * c : DS * (c + 1)] = hc[:, :, ::-1].transpose(2, 1, 0)
    return out.reshape(T * B, D), res


def kernel(f, x):
    return _run(f, x, trace=False)[0]
